# revision 1
# baseline (speedup 1.0000x reference)
"""Block-diagonal linear for TRN2, 8 NeuronCores.

y = concat_h(x_h @ w_h + b_h) with x:[4,4096,4096] split into 16 blocks of
256 features; w:[16,256,256]; b:[16,256].

Sharding: data-parallel over rows. x is reshaped to [16384, 4096] and each
core takes 2048 contiguous rows; w and b are replicated. Zero communication.

Per-core kernel (Tile framework):
  - w is staged in SBUF as [128, 16, 2, 256] (contraction dim on partitions).
  - b is broadcast across partitions once via gpsimd partition_broadcast.
  - For each 128-row tile of x: DMA in naturally (4 chunks), then per group of
    4 feature-chunks: PE-transpose them (features onto partitions, bit-exact
    fp32 transpose via identity matmul), copy PSUM->SBUF on ACT, and run the
    2-step accumulated fp32 matmuls for the 2 feature blocks they feed; DVE
    adds the bias while evicting PSUM->SBUF; y DMAs out in 4 chunks.
  - Everything fp32: exact same arithmetic as the reference (PE fp32 matmul
    is bit-accurate per-product with fp32 PSUM accumulation).
"""

import numpy as np

import concourse.bacc as bacc
import concourse.mybir as mybir
from concourse import bass2jax, tile
from concourse.masks import make_identity

N_CORES = 8
ROWS_TOTAL = 4 * 4096
ROWS = ROWS_TOTAL // N_CORES  # 2048 rows per core
WIDTH = 4096
NB = 16  # feature blocks
BW = 256  # block width
P = 128
M_TILES = ROWS // P  # 16

FP32 = mybir.dt.float32


def _build(repeat=1, xb=2, xtb=2, yb=2, ptb=3, pyb=5):
    nc = bacc.Bacc(None, target_bir_lowering=False, debug=False)
    x = nc.dram_tensor("x", [ROWS, WIDTH], FP32, kind="ExternalInput")
    w = nc.dram_tensor("w", [NB, BW, BW], FP32, kind="ExternalInput")
    b = nc.dram_tensor("b", [NB, BW], FP32, kind="ExternalInput")
    y = nc.dram_tensor("y", [ROWS, WIDTH], FP32, kind="ExternalOutput")

    with tile.TileContext(nc) as tc:
        with (
            tc.tile_pool(name="const", bufs=1) as const_pool,
            tc.tile_pool(name="xpool", bufs=xb) as x_pool,
            tc.tile_pool(name="xtpool", bufs=xtb) as xt_pool,
            tc.tile_pool(name="ypool", bufs=yb) as y_pool,
            tc.tile_pool(name="pt", bufs=ptb, space="PSUM") as psum_t,
            tc.tile_pool(name="py", bufs=pyb, space="PSUM") as psum_y,
        ):
            ident = const_pool.tile([P, P], FP32)
            make_identity(nc, ident[:])

            # Weights + bias on the ACT HWDGE ring so they don't head-block
            # the x-tile loads issued on the SP ring.
            # w_sb[p, h, ic, j] = w[h, ic*128+p, j]; staged in 4 chunks so the
            # first matmuls only wait for blocks 0-3.
            w_sb = const_pool.tile([P, NB, 2, BW], FP32)
            w_re = w.rearrange("h (ic p) j -> p h ic j", p=P)
            for q in range(4):
                nc.scalar.dma_start(
                    w_sb[:, 4 * q:4 * q + 4], w_re[:, 4 * q:4 * q + 4]
                )

            # Bias broadcast across partitions on GpSimd (Pool engine):
            # b_rep[p, h, j] = b[h, j].
            b_lin = const_pool.tile([1, NB, BW], FP32)
            nc.scalar.dma_start(
                b_lin[:], b.rearrange("(one h) j -> one h j", one=1)
            )
            b_rep = const_pool.tile([P, NB, BW], FP32)
            nc.gpsimd.partition_broadcast(
                b_rep[:].rearrange("p h j -> p (h j)"),
                b_lin[:].rearrange("o h j -> o (h j)"),
            )

            x_rows = x.rearrange("(t p) i -> t p i", p=P)
            y_rows = y.rearrange("(t p) i -> t p i", p=P)

            import contextlib

            rep_ctx = (
                tc.For_i(0, repeat, 1) if repeat > 1 else contextlib.nullcontext()
            )
            with rep_ctx:
                _main_loop(nc, tc, locals())

    nc.compile()
    return nc


def _main_loop(nc, tc, env):
    x_pool = env["x_pool"]
    xt_pool = env["xt_pool"]
    y_pool = env["y_pool"]
    psum_t = env["psum_t"]
    psum_y = env["psum_y"]
    ident = env["ident"]
    w_sb = env["w_sb"]
    b_rep = env["b_rep"]
    x_rows = env["x_rows"]
    y_rows = env["y_rows"]
    if True:
            for mi in range(M_TILES):
                x_t = x_pool.tile([P, WIDTH], FP32)
                qw = WIDTH // 4
                for q in range(4):
                    nc.sync.dma_start(
                        x_t[:, q * qw:(q + 1) * qw],
                        x_rows[mi][:, q * qw:(q + 1) * qw],
                    )

                # Per group g: transpose chunks 4g..4g+3 (features onto
                # partitions), then immediately the matmuls for blocks
                # 2g, 2g+1 which consume exactly those chunks. Interleaving
                # keeps real matmuls inside every HAM activity window.
                xT = xt_pool.tile([P, WIDTH // P, P], FP32)
                y_t = y_pool.tile([P, NB, BW], FP32)
                for g in range(8):
                    pt = psum_t.tile([P, 4, P], FP32, tag="pt")
                    for k in range(4):
                        c = 4 * g + k
                        nc.tensor.transpose(
                            pt[:, k, :], x_t[:, c * P:(c + 1) * P], ident[:]
                        )
                    nc.scalar.copy(xT[:, 4 * g:4 * g + 4, :], pt[:])

                    py = psum_y.tile([P, 2, BW], FP32)
                    for u in range(2):
                        h = 2 * g + u
                        nc.tensor.matmul(
                            py[:, u, :], xT[:, 2 * h, :], w_sb[:, h, 0, :],
                            start=True, stop=False,
                        )
                        nc.tensor.matmul(
                            py[:, u, :], xT[:, 2 * h + 1, :], w_sb[:, h, 1, :],
                            start=False, stop=True,
                        )
                    nc.vector.tensor_add(
                        y_t[:, 2 * g:2 * g + 2, :],
                        py[:],
                        b_rep[:, 2 * g:2 * g + 2, :],
                    )
                    if mi == M_TILES - 1:
                        # Last tile: stream each group's slice out right after
                        # its bias-add so the kernel tail is just one small
                        # DMA instead of a quarter-row.
                        nc.sync.dma_start(
                            y_rows[mi][:, g * 512:(g + 1) * 512],
                            y_t[:].rearrange("p h j -> p (h j)")[
                                :, g * 512:(g + 1) * 512
                            ],
                        )
                if mi != M_TILES - 1:
                    y_flat = y_t[:].rearrange("p h j -> p (h j)")
                    qw = WIDTH // 4
                    for q in range(4):
                        nc.sync.dma_start(
                            y_rows[mi][:, q * qw:(q + 1) * qw],
                            y_flat[:, q * qw:(q + 1) * qw],
                        )


class _Runner:
    """Compile once, keep the jitted SPMD executable for reuse."""

    def __init__(self, repeat=1):
        import jax
        from jax.experimental.shard_map import shard_map
        from jax.sharding import Mesh, PartitionSpec

        self.jax = jax
        nc = _build(repeat=repeat)
        bass2jax.install_neuronx_cc_hook()

        assert nc.dbg_addr is None
        part_name = (
            nc.partition_id_tensor.name if nc.partition_id_tensor else None
        )
        in_names, out_names, out_avals = [], [], []
        for alloc in nc.m.functions[0].allocations:
            if not isinstance(alloc, mybir.MemoryLocationSet):
                continue
            name = alloc.memorylocations[0].name
            if alloc.kind == "ExternalInput":
                if name != part_name:
                    in_names.append(name)
            elif alloc.kind == "ExternalOutput":
                out_names.append(name)
                out_avals.append(
                    jax.core.ShapedArray(
                        tuple(alloc.tensor_shape), mybir.dt.np(alloc.dtype)
                    )
                )
        self.in_names = list(in_names)
        self.out_names = out_names
        self.out_avals = out_avals
        n_params = len(in_names)
        n_outs = len(out_names)
        all_names = in_names + out_names
        if part_name is not None:
            all_names = all_names + [part_name]

        def _body(*args):
            operands = list(args)
            if part_name is not None:
                operands.append(bass2jax.partition_id_tensor())
            outs = bass2jax._bass_exec_p.bind(
                *operands,
                out_avals=tuple(out_avals),
                in_names=tuple(all_names),
                out_names=tuple(out_names),
                lowering_input_output_aliases=(),
                sim_require_finite=True,
                sim_require_nnan=True,
                nc=nc,
            )
            return tuple(outs)

        devices = jax.devices()[:N_CORES]
        assert len(devices) == N_CORES
        self.mesh = Mesh(np.asarray(devices), ("core",))
        in_specs = (PartitionSpec("core"),) * (n_params + n_outs)
        out_specs = (PartitionSpec("core"),) * n_outs
        self.donate = tuple(range(n_params, n_params + n_outs))
        self.fn = jax.jit(
            shard_map(
                _body,
                mesh=self.mesh,
                in_specs=in_specs,
                out_specs=out_specs,
                check_rep=False,
            ),
            donate_argnums=self.donate,
            keep_unused=True,
        )

    def zeros(self):
        return [
            np.zeros((N_CORES * a.shape[0], *a.shape[1:]), a.dtype)
            for a in self.out_avals
        ]

    def prep(self, x, w, b):
        """Global (concatenated-over-cores) input arrays, in in_names order."""
        x2 = np.ascontiguousarray(
            np.asarray(x, dtype=np.float32).reshape(ROWS_TOTAL, WIDTH)
        )
        w = np.ascontiguousarray(np.asarray(w, dtype=np.float32))
        b = np.ascontiguousarray(np.asarray(b, dtype=np.float32))
        per = {
            "x": x2,
            "w": np.concatenate([w] * N_CORES, axis=0),
            "b": np.concatenate([b] * N_CORES, axis=0),
        }
        return [per[n] for n in self.in_names]

    def __call__(self, ins, zeros):
        outs = self.fn(*ins, *zeros)
        return dict(zip(self.out_names, outs))


_RUNNER = None


def _get_runner():
    global _RUNNER
    if _RUNNER is None:
        _RUNNER = _Runner()
    return _RUNNER


def kernel(x, w, b):
    r = _get_runner()
    outs = r(r.prep(x, w, b), r.zeros())
    y = np.asarray(outs["y"])
    return y.reshape(4, 4096, WIDTH)



# revision 3
# speedup vs baseline: 409.7234x; 409.7234x over previous
"""Block-diagonal linear for TRN2, 8 NeuronCores.

y = concat_h(x_h @ w_h + b_h) with x:[4,4096,4096] split into 16 blocks of
256 features; w:[16,256,256]; b:[16,256].

Sharding: data-parallel over rows. x is reshaped to [16384, 4096] and each
core takes 2048 contiguous rows; w and b are replicated. Zero communication.

Per-core kernel (Tile framework), fp16 data path:
  - x and w are staged in DRAM as fp16 (converted host-side); products are
    accumulated in fp32 PSUM, so the only precision loss is input/output
    quantization (~1e-3 rel worst-case, well inside the 2e-2 gate).
  - fp16 halves HBM traffic vs fp32 and runs the PE at 1 cycle/row for both
    the transposes and the matmuls (fp32 matmul costs 4 cycles/row).
  - w is staged in SBUF as [128, 16, 2, 256] (contraction dim on partitions).
  - b is broadcast across partitions once via gpsimd partition_broadcast.
  - For each 128-row tile of x: DMA in naturally (4 chunks), then per group of
    4 feature-chunks: PE-transpose them (features onto partitions), copy
    PSUM->SBUF on ACT (downcast to fp16), and run the 2-step fp32-accumulated
    matmuls for the 2 feature blocks they feed; DVE adds the bias while
    evicting PSUM->SBUF (fp16); y DMAs out in 4 chunks.
  - The whole main loop sits in a hardware For_i whose trip count comes from
    a tiny "reps" input tensor: reps=1 for normal execution, reps=R for
    R-loop delta timing (same NEFF, no recompile).
"""

import numpy as np

import concourse.bacc as bacc
import concourse.mybir as mybir
from concourse import bass2jax, tile
from concourse.masks import make_identity

N_CORES = 8
ROWS_TOTAL = 4 * 4096
ROWS = ROWS_TOTAL // N_CORES  # 2048 rows per core
WIDTH = 4096
NB = 16  # feature blocks
BW = 256  # block width
P = 128
M_TILES = ROWS // P  # 16

FP32 = mybir.dt.float32
FP16 = mybir.dt.float16
INT32 = mybir.dt.int32


def _build(xb=2, xtb=2, yb=2, ptb=3, pyb=5):
    nc = bacc.Bacc(None, target_bir_lowering=False, debug=False)
    x = nc.dram_tensor("x", [ROWS, WIDTH], FP16, kind="ExternalInput")
    w = nc.dram_tensor("w", [NB, BW, BW], FP16, kind="ExternalInput")
    b = nc.dram_tensor("b", [NB, BW], FP32, kind="ExternalInput")
    reps = nc.dram_tensor("reps", [1, 1], INT32, kind="ExternalInput")
    y = nc.dram_tensor("y", [ROWS, WIDTH], FP16, kind="ExternalOutput")

    with tile.TileContext(nc) as tc:
        with (
            tc.tile_pool(name="const", bufs=1) as const_pool,
            tc.tile_pool(name="xpool", bufs=xb) as x_pool,
            tc.tile_pool(name="xtpool", bufs=xtb) as xt_pool,
            tc.tile_pool(name="ypool", bufs=yb) as y_pool,
            tc.tile_pool(name="pt", bufs=ptb, space="PSUM") as psum_t,
            tc.tile_pool(name="py", bufs=pyb, space="PSUM") as psum_y,
        ):
            ident = const_pool.tile([P, P], FP16)
            make_identity(nc, ident[:])

            # Weights + bias on the ACT HWDGE ring so they don't head-block
            # the x-tile loads issued on the SP ring.
            # w_sb[p, h, ic, j] = w[h, ic*128+p, j]; staged in 4 chunks so the
            # first matmuls only wait for blocks 0-3.
            w_sb = const_pool.tile([P, NB, 2, BW], FP16)
            w_re = w.rearrange("h (ic p) j -> p h ic j", p=P)
            for q in range(4):
                nc.scalar.dma_start(
                    w_sb[:, 4 * q:4 * q + 4], w_re[:, 4 * q:4 * q + 4]
                )

            # Bias broadcast across partitions on GpSimd (Pool engine):
            # b_rep[p, h, j] = b[h, j].
            b_lin = const_pool.tile([1, NB, BW], FP32)
            nc.scalar.dma_start(
                b_lin[:], b.rearrange("(one h) j -> one h j", one=1)
            )
            b_rep = const_pool.tile([P, NB, BW], FP32)
            nc.gpsimd.partition_broadcast(
                b_rep[:].rearrange("p h j -> p (h j)"),
                b_lin[:].rearrange("o h j -> o (h j)"),
            )

            # Dynamic repeat count (1 for normal runs, R for delta timing).
            r_sb = const_pool.tile([1, 1], INT32)
            nc.sync.dma_start(r_sb[:], reps[:])
            rv = nc.values_load(
                r_sb[:], min_val=1, max_val=1 << 20,
                skip_runtime_bounds_check=True,
            )

            x_rows = x.rearrange("(t p) i -> t p i", p=P)
            y_rows = y.rearrange("(t p) i -> t p i", p=P)

            with tc.For_i(0, rv, 1):
                for mi in range(M_TILES):
                    x_t = x_pool.tile([P, WIDTH], FP16)
                    qw = WIDTH // 4
                    for q in range(4):
                        nc.sync.dma_start(
                            x_t[:, q * qw:(q + 1) * qw],
                            x_rows[mi][:, q * qw:(q + 1) * qw],
                        )

                    # Per group g: transpose chunks 4g..4g+3 (features onto
                    # partitions), then immediately the matmuls for blocks
                    # 2g, 2g+1 which consume exactly those chunks.
                    xT = xt_pool.tile([P, WIDTH // P, P], FP16)
                    y_t = y_pool.tile([P, NB, BW], FP16)
                    for g in range(8):
                        pt = psum_t.tile([P, 4, P], FP16, tag="pt")
                        for k in range(4):
                            c = 4 * g + k
                            nc.tensor.transpose(
                                pt[:, k, :], x_t[:, c * P:(c + 1) * P], ident[:]
                            )
                        nc.scalar.copy(xT[:, 4 * g:4 * g + 4, :], pt[:])

                        py = psum_y.tile([P, 2, BW], FP32)
                        for u in range(2):
                            h = 2 * g + u
                            nc.tensor.matmul(
                                py[:, u, :], xT[:, 2 * h, :], w_sb[:, h, 0, :],
                                start=True, stop=False,
                            )
                            nc.tensor.matmul(
                                py[:, u, :], xT[:, 2 * h + 1, :], w_sb[:, h, 1, :],
                                start=False, stop=True,
                            )
                        nc.vector.tensor_add(
                            y_t[:, 2 * g:2 * g + 2, :],
                            py[:],
                            b_rep[:, 2 * g:2 * g + 2, :],
                        )
                        if mi == M_TILES - 1:
                            # Last tile: stream each group's slice out right
                            # after its bias-add so the kernel tail is just
                            # one small DMA instead of a quarter-row.
                            nc.sync.dma_start(
                                y_rows[mi][:, g * 512:(g + 1) * 512],
                                y_t[:].rearrange("p h j -> p (h j)")[
                                    :, g * 512:(g + 1) * 512
                                ],
                            )
                    if mi != M_TILES - 1:
                        y_flat = y_t[:].rearrange("p h j -> p (h j)")
                        qw = WIDTH // 4
                        for q in range(4):
                            nc.sync.dma_start(
                                y_rows[mi][:, q * qw:(q + 1) * qw],
                                y_flat[:, q * qw:(q + 1) * qw],
                            )

    nc.compile()
    return nc


class _Runner:
    """Compile once, keep the jitted SPMD executable for reuse."""

    def __init__(self):
        import jax
        from jax.experimental.shard_map import shard_map
        from jax.sharding import Mesh, PartitionSpec

        self.jax = jax
        nc = _build()
        bass2jax.install_neuronx_cc_hook()

        assert nc.dbg_addr is None
        part_name = (
            nc.partition_id_tensor.name if nc.partition_id_tensor else None
        )
        in_names, out_names, out_avals = [], [], []
        for alloc in nc.m.functions[0].allocations:
            if not isinstance(alloc, mybir.MemoryLocationSet):
                continue
            name = alloc.memorylocations[0].name
            if alloc.kind == "ExternalInput":
                if name != part_name:
                    in_names.append(name)
            elif alloc.kind == "ExternalOutput":
                out_names.append(name)
                out_avals.append(
                    jax.core.ShapedArray(
                        tuple(alloc.tensor_shape), mybir.dt.np(alloc.dtype)
                    )
                )
        self.in_names = list(in_names)
        self.out_names = out_names
        self.out_avals = out_avals
        n_params = len(in_names)
        all_names = list(in_names)
        if part_name is not None:
            all_names = all_names + [part_name]

        def _body(*args):
            operands = list(args)
            if part_name is not None:
                operands.append(bass2jax.partition_id_tensor())
            outs = bass2jax._bass_exec_p.bind(
                *operands,
                out_avals=tuple(out_avals),
                in_names=tuple(all_names),
                out_names=tuple(out_names),
                lowering_input_output_aliases=(),
                sim_require_finite=True,
                sim_require_nnan=True,
                nc=nc,
            )
            return tuple(outs)

        devices = jax.devices()[:N_CORES]
        assert len(devices) == N_CORES
        self.mesh = Mesh(np.asarray(devices), ("core",))
        in_specs = (PartitionSpec("core"),) * n_params
        out_specs = (PartitionSpec("core"),) * len(out_names)
        self.fn = jax.jit(
            shard_map(
                _body,
                mesh=self.mesh,
                in_specs=in_specs,
                out_specs=out_specs,
                check_rep=False,
            ),
            keep_unused=True,
        )

    def prep(self, x, w, b, reps=1):
        """Global (concatenated-over-cores) input arrays, in in_names order."""
        x2 = np.ascontiguousarray(
            np.asarray(x).reshape(ROWS_TOTAL, WIDTH).astype(np.float16)
        )
        w16 = np.ascontiguousarray(np.asarray(w, dtype=np.float16))
        b32 = np.ascontiguousarray(np.asarray(b, dtype=np.float32))
        per = {
            "x": x2,
            "w": np.concatenate([w16] * N_CORES, axis=0),
            "b": np.concatenate([b32] * N_CORES, axis=0),
            "reps": np.full((N_CORES, 1), reps, np.int32),
        }
        return [per[n] for n in self.in_names]

    def __call__(self, ins):
        outs = self.fn(*ins)
        return dict(zip(self.out_names, outs))


_RUNNER = None


def _get_runner():
    global _RUNNER
    if _RUNNER is None:
        _RUNNER = _Runner()
    return _RUNNER


def kernel(x, w, b):
    r = _get_runner()
    outs = r(r.prep(x, w, b))
    y = np.asarray(outs["y"])
    return y.astype(np.float32).reshape(4, 4096, WIDTH)


# revision 7
# speedup vs baseline: 517.6146x; 1.2633x over previous
"""Block-diagonal linear for TRN2, 8 NeuronCores.

y = concat_h(x_h @ w_h + b_h) with x:[4,4096,4096] split into 16 blocks of
256 features; w:[16,256,256]; b:[16,256].

Sharding: data-parallel over rows. x is reshaped to [16384, 4096] and each
core takes 2048 contiguous rows; w and b are replicated. Zero communication.

Per-core kernel (Tile framework), fp16 data path:
  - x and w are staged in DRAM as fp16 (converted host-side); products are
    accumulated in fp32 PSUM, so the only precision loss is input/output
    quantization (~1e-3 rel worst-case, well inside the 2e-2 gate).
  - fp16 halves HBM traffic vs fp32 and runs the PE at 1 cycle/row for both
    the transposes and the matmuls (fp32 matmul costs 4 cycles/row).
  - w is staged in SBUF as [128, 16, 2, 256] (contraction dim on partitions).
  - b is broadcast across partitions once via gpsimd partition_broadcast.
  - For each 128-row tile of x: DMA in naturally (4 chunks), then per group of
    4 feature-chunks: PE-transpose them (features onto partitions), copy
    PSUM->SBUF on ACT (downcast to fp16), and run the 2-step fp32-accumulated
    matmuls for the 2 feature blocks they feed; DVE adds the bias while
    evicting PSUM->SBUF (fp16); y DMAs out in 4 chunks.
  - The whole main loop sits in a hardware For_i whose trip count comes from
    a tiny "reps" input tensor: reps=1 for normal execution, reps=R for
    R-loop delta timing (same NEFF, no recompile).
"""

import numpy as np

import concourse.bacc as bacc
import concourse.mybir as mybir
from concourse import bass2jax, tile
from concourse.masks import make_identity

N_CORES = 8
ROWS_TOTAL = 4 * 4096
ROWS = ROWS_TOTAL // N_CORES  # 2048 rows per core
WIDTH = 4096
NB = 16  # feature blocks
BW = 256  # block width
P = 128
M_TILES = ROWS // P  # 16

FP32 = mybir.dt.float32
FP16 = mybir.dt.float16
INT32 = mybir.dt.int32


def _build(xb=3, xtb=2, yb=2, ptb=3, pyb=5):
    nc = bacc.Bacc(None, target_bir_lowering=False, debug=False)
    x = nc.dram_tensor("x", [ROWS, WIDTH], FP16, kind="ExternalInput")
    w = nc.dram_tensor("w", [NB, BW, BW], FP16, kind="ExternalInput")
    b = nc.dram_tensor("b", [NB, BW], FP32, kind="ExternalInput")
    reps = nc.dram_tensor("reps", [1, 1], INT32, kind="ExternalInput")
    y = nc.dram_tensor("y", [ROWS, WIDTH], FP16, kind="ExternalOutput")

    with tile.TileContext(nc) as tc:
        with (
            tc.tile_pool(name="const", bufs=1) as const_pool,
            tc.tile_pool(name="xpool", bufs=xb) as x_pool,
            tc.tile_pool(name="xtpool", bufs=xtb) as xt_pool,
            tc.tile_pool(name="ypool", bufs=yb) as y_pool,
            tc.tile_pool(name="pt", bufs=ptb, space="PSUM") as psum_t,
            tc.tile_pool(name="py", bufs=pyb, space="PSUM") as psum_y,
        ):
            ident = const_pool.tile([P, P], FP16)
            make_identity(nc, ident[:])

            # Weights + bias on the ACT HWDGE ring so they don't head-block
            # the x-tile loads issued on the SP ring.
            # w_sb[p, h, ic, j] = w[h, ic*128+p, j]; staged in 4 chunks so the
            # first matmuls only wait for blocks 0-3.
            w_sb = const_pool.tile([P, NB, 2, BW], FP16)
            w_re = w.rearrange("h (ic p) j -> p h ic j", p=P)
            for q in range(4):
                nc.scalar.dma_start(
                    w_sb[:, 4 * q:4 * q + 4], w_re[:, 4 * q:4 * q + 4]
                )

            # Bias broadcast across partitions on GpSimd (Pool engine):
            # b_rep[p, h, j] = b[h, j].
            b_lin = const_pool.tile([1, NB, BW], FP32)
            nc.scalar.dma_start(
                b_lin[:], b.rearrange("(one h) j -> one h j", one=1)
            )
            b_rep = const_pool.tile([P, NB, BW], FP32)
            nc.gpsimd.partition_broadcast(
                b_rep[:].rearrange("p h j -> p (h j)"),
                b_lin[:].rearrange("o h j -> o (h j)"),
            )

            # Dynamic repeat count (1 for normal runs, R for delta timing).
            r_sb = const_pool.tile([1, 1], INT32)
            nc.sync.dma_start(r_sb[:], reps[:])
            rv = nc.values_load(
                r_sb[:], min_val=1, max_val=1 << 20,
                skip_runtime_bounds_check=True,
            )

            x_rows = x.rearrange("(t p) i -> t p i", p=P)
            y_rows = y.rearrange("(t p) i -> t p i", p=P)

            with tc.For_i(0, rv, 1):
                for mi in range(M_TILES):
                    x_t = x_pool.tile([P, WIDTH], FP16)
                    qw = WIDTH // 4
                    for q in range(4):
                        nc.sync.dma_start(
                            x_t[:, q * qw:(q + 1) * qw],
                            x_rows[mi][:, q * qw:(q + 1) * qw],
                        )

                    # Group g = feature chunks 4g..4g+3 = blocks 2g, 2g+1.
                    # The PE is in-order, so transposes run two groups ahead
                    # of the matmuls: by the time matmuls(g) issue, the ACT
                    # copy of xT(g) finished long ago and the PE never stalls
                    # on it.
                    xT = xt_pool.tile([P, WIDTH // P, P], FP16)
                    y_t = y_pool.tile([P, NB, BW], FP16)

                    def transpose_group(g, x_t=x_t, xT=xT):
                        pt = psum_t.tile([P, 4, P], FP16, tag="pt")
                        for k in range(4):
                            c = 4 * g + k
                            nc.tensor.transpose(
                                pt[:, k, :], x_t[:, c * P:(c + 1) * P], ident[:]
                            )
                        nc.scalar.copy(xT[:, 4 * g:4 * g + 4, :], pt[:])

                    transpose_group(0)
                    transpose_group(1)
                    for g in range(8):
                        if g + 2 < 8:
                            transpose_group(g + 2)
                        py = psum_y.tile([P, 2, BW], FP32)
                        for u in range(2):
                            h = 2 * g + u
                            nc.tensor.matmul(
                                py[:, u, :], xT[:, 2 * h, :], w_sb[:, h, 0, :],
                                start=True, stop=False,
                            )
                            nc.tensor.matmul(
                                py[:, u, :], xT[:, 2 * h + 1, :], w_sb[:, h, 1, :],
                                start=False, stop=True,
                            )
                        nc.vector.tensor_add(
                            y_t[:, 2 * g:2 * g + 2, :],
                            py[:],
                            b_rep[:, 2 * g:2 * g + 2, :],
                        )
                        if mi == M_TILES - 1:
                            # Last tile: stream each group's slice out right
                            # after its bias-add so the kernel tail is just
                            # one small DMA instead of a quarter-row.
                            nc.gpsimd.dma_start(
                                y_rows[mi][:, g * 512:(g + 1) * 512],
                                y_t[:].rearrange("p h j -> p (h j)")[
                                    :, g * 512:(g + 1) * 512
                                ],
                            )
                    if mi != M_TILES - 1:
                        # y stores ride the Pool (gpsimd) ring so they don't
                        # contend with the x loads on the SP ring.
                        y_flat = y_t[:].rearrange("p h j -> p (h j)")
                        qw = WIDTH // 4
                        for q in range(4):
                            nc.gpsimd.dma_start(
                                y_rows[mi][:, q * qw:(q + 1) * qw],
                                y_flat[:, q * qw:(q + 1) * qw],
                            )

    nc.compile()
    return nc


class _Runner:
    """Compile once, keep the jitted SPMD executable for reuse."""

    def __init__(self):
        import jax
        from jax.experimental.shard_map import shard_map
        from jax.sharding import Mesh, PartitionSpec

        self.jax = jax
        nc = _build()
        bass2jax.install_neuronx_cc_hook()

        assert nc.dbg_addr is None
        part_name = (
            nc.partition_id_tensor.name if nc.partition_id_tensor else None
        )
        in_names, out_names, out_avals = [], [], []
        for alloc in nc.m.functions[0].allocations:
            if not isinstance(alloc, mybir.MemoryLocationSet):
                continue
            name = alloc.memorylocations[0].name
            if alloc.kind == "ExternalInput":
                if name != part_name:
                    in_names.append(name)
            elif alloc.kind == "ExternalOutput":
                out_names.append(name)
                out_avals.append(
                    jax.core.ShapedArray(
                        tuple(alloc.tensor_shape), mybir.dt.np(alloc.dtype)
                    )
                )
        self.in_names = list(in_names)
        self.out_names = out_names
        self.out_avals = out_avals
        n_params = len(in_names)
        all_names = list(in_names)
        if part_name is not None:
            all_names = all_names + [part_name]

        def _body(*args):
            operands = list(args)
            if part_name is not None:
                operands.append(bass2jax.partition_id_tensor())
            outs = bass2jax._bass_exec_p.bind(
                *operands,
                out_avals=tuple(out_avals),
                in_names=tuple(all_names),
                out_names=tuple(out_names),
                lowering_input_output_aliases=(),
                sim_require_finite=True,
                sim_require_nnan=True,
                nc=nc,
            )
            return tuple(outs)

        devices = jax.devices()[:N_CORES]
        assert len(devices) == N_CORES
        self.mesh = Mesh(np.asarray(devices), ("core",))
        in_specs = (PartitionSpec("core"),) * n_params
        out_specs = (PartitionSpec("core"),) * len(out_names)
        self.fn = jax.jit(
            shard_map(
                _body,
                mesh=self.mesh,
                in_specs=in_specs,
                out_specs=out_specs,
                check_rep=False,
            ),
            keep_unused=True,
        )

    def prep(self, x, w, b, reps=1):
        """Global (concatenated-over-cores) input arrays, in in_names order."""
        x2 = np.ascontiguousarray(
            np.asarray(x).reshape(ROWS_TOTAL, WIDTH).astype(np.float16)
        )
        w16 = np.ascontiguousarray(np.asarray(w, dtype=np.float16))
        b32 = np.ascontiguousarray(np.asarray(b, dtype=np.float32))
        per = {
            "x": x2,
            "w": np.concatenate([w16] * N_CORES, axis=0),
            "b": np.concatenate([b32] * N_CORES, axis=0),
            "reps": np.full((N_CORES, 1), reps, np.int32),
        }
        return [per[n] for n in self.in_names]

    def __call__(self, ins):
        outs = self.fn(*ins)
        return dict(zip(self.out_names, outs))


_RUNNER = None


def _get_runner():
    global _RUNNER
    if _RUNNER is None:
        _RUNNER = _Runner()
    return _RUNNER


def kernel(x, w, b):
    r = _get_runner()
    outs = r(r.prep(x, w, b))
    y = np.asarray(outs["y"])
    return y.astype(np.float32).reshape(4, 4096, WIDTH)
